# revision 16
# baseline (speedup 1.0000x reference)
"""Trainium2 Bass kernel for the ControlUnit problem.

Computation (per batch b):
    cq      = concat([control_state, question])            # [2D]
    cq_proj = cq @ W_cq + b_cq + step_emb[step]            # [D]
    qw      = cq_proj * W_attn                             # [D]
    logits  = context[b] @ qw  (+ b_attn, softmax-invariant)
    w       = softmax(where(mask, logits, -inf))           # [L]
    out[b]  = w @ context[b]                               # [D]

Sharding: data-parallel over batch across 8 NeuronCores (8 batches/core).
All params replicated (W_cq cast to bf16 to halve its DMA traffic).

Per-core kernel structure:
  Phase 1: cq_proj matmul (bias folded in via an augmented ones-row of cq /
           bias-row of W), qw = cq_proj * W_attn, PE-transpose qw -> qwT
           so d sits on partitions.
  Phase 2 (per batch): stream context tiles [128l, D] as bf16 (cast in the
           DMA), PE-transpose them into [128d, l] chunks, matmul qwT against
           the transposed chunks to get logits, exp (no max subtraction:
           logits are ~N(0,1)), mask, PE-transpose the exp-weights into
           l-on-partition columns, then accumulate the weighted sum of the
           natural-layout tiles into a shared PSUM tile (per-batch weight
           columns are zero-padded so all 8 batches share one [8, D] PSUM
           accumulator).  The softmax denominator comes from a ones-matmul
           and is divided out once at the end.
"""

import numpy as np
import ml_dtypes
from contextlib import ExitStack

import concourse.bass as bass
import concourse.tile as tile
from concourse import bacc, mybir
from concourse.bass_utils import run_bass_kernel_spmd

F32 = mybir.dt.float32
BF16 = mybir.dt.bfloat16

N_CORES = 8
B, L, D = 64, 512, 2048


def build_nc(b_c, l, d, n_cores):
    """Build + compile the per-core Bass program (SPMD: same program on all
    cores, different data)."""
    b_full = b_c * n_cores     # global batch
    k_sh = (2 * d) // n_cores + 128   # per-core K shard + bias row block
    KT = k_sh // 128           # k-tiles for the cq_proj matmul
    LT = l // 128              # l-tiles per batch
    DC = d // 128              # 128-wide d-chunks
    NN = d // 512              # 512-wide n-chunks
    CT_G = min(8, DC)          # transposed chunks per PSUM group
    NG = DC // CT_G

    nc = bacc.Bacc("TRN2", target_bir_lowering=False, debug=False,
                   num_devices=n_cores)

    ctx_d = nc.dram_tensor("ctx", [b_c, l, d], F32, kind="ExternalInput")
    cqT_d = nc.dram_tensor("cqT", [128, KT, b_full], BF16, kind="ExternalInput")
    w_d = nc.dram_tensor("w_aug", [k_sh, d], BF16, kind="ExternalInput")
    cqp_all_d = nc.dram_tensor("cqp_all", [b_full, d], F32)
    cqp_rs_d = nc.dram_tensor("cqp_rs", [b_c, d], F32)
    wattn_d = nc.dram_tensor("wattn", [b_c, d], F32, kind="ExternalInput")
    mask_d = nc.dram_tensor("mask", [b_c, l], F32, kind="ExternalInput")
    idb_d = nc.dram_tensor("ident_bf16", [128, 128], BF16, kind="ExternalInput")
    ones_d = nc.dram_tensor("ones_col", [128, 1], BF16, kind="ExternalInput")
    out_d = nc.dram_tensor("out", [b_c, d], F32, kind="ExternalOutput")

    Exp = mybir.ActivationFunctionType.Exp

    with tile.TileContext(nc) as tc:
        with ExitStack() as ctx:
            const = ctx.enter_context(tc.tile_pool(name="const", bufs=1))
            wpool = ctx.enter_context(tc.tile_pool(name="wpool", bufs=3))
            natpool = ctx.enter_context(tc.tile_pool(name="natpool", bufs=3 * LT))
            ctpool = ctx.enter_context(tc.tile_pool(name="ctpool", bufs=3))
            ps_big = ctx.enter_context(tc.tile_pool(name="ps_big", bufs=1, space="PSUM"))
            ps_misc_p = ctx.enter_context(tc.tile_pool(name="ps_misc_p", bufs=1, space="PSUM"))
            ps_lg_p = ctx.enter_context(tc.tile_pool(name="ps_lg_p", bufs=1, space="PSUM"))
            lgpool = ctx.enter_context(tc.tile_pool(name="lgpool", bufs=2))
            ps_ct_p = ctx.enter_context(tc.tile_pool(name="ps_ct_p", bufs=2, space="PSUM"))

            # ---- constants / persistent tiles ----
            idb = const.tile([128, 128], BF16)
            nc.sync.dma_start(idb[:, :], idb_d[:, :])
            ones_sb = const.tile([128, 1], BF16)
            nc.sync.dma_start(ones_sb[:, :], ones_d[:, :])
            cqT_sb = const.tile([128, KT, b_full], BF16)
            nc.sync.dma_start(cqT_sb[:, :, :], cqT_d[:, :, :])
            wattn_sb = const.tile([b_c, d], F32)
            nc.sync.dma_start(wattn_sb[:, :], wattn_d[:, :])
            mask_sb = const.tile([b_c, l], F32)
            nc.sync.dma_start(mask_sb[:, :], mask_d[:, :])

            cqp_sb = const.tile([b_full, d], F32)
            cqp_my = const.tile([b_c, d], F32)
            qw_sb = const.tile([b_c, d], BF16)
            qwT_sb = const.tile([128, DC * b_c], BF16)
            W8 = const.tile([128, b_c, LT, b_c], BF16)
            nc.gpsimd.memset(W8[:, :, :, :], 0.0)
            sinv_sb = const.tile([b_c, 1], F32)
            out_sb = const.tile([b_c, d], F32)

            # ---- PSUM tiles ----
            # ps_bigt: partial cq_proj accumulator ([b_full, d]) in phase 1,
            # output accumulator (first b_c rows) in phase 2; one allocation
            # keeps us within the 8 PSUM banks.
            ps_bigt = ps_big.tile([b_full, d], F32)
            # dedicated bank for the softmax denominator accumulation group
            ps_s = ps_misc_p.tile([b_c, 1], F32)

            # ---- phase 1: partial cq_proj for ALL batches (K shard),
            #      ReduceScatter over cores -> this core's b_c rows ----
            for k in range(KT):
                wk = wpool.tile([128, d], BF16)
                nc.sync.dma_start(wk[:, :], w_d[k * 128:(k + 1) * 128, :])
                for n in range(NN):
                    nc.tensor.matmul(
                        ps_bigt[:, n * 512:(n + 1) * 512],
                        lhsT=cqT_sb[:, k, :],
                        rhs=wk[:, n * 512:(n + 1) * 512],
                        start=(k == 0),
                        stop=(k == KT - 1),
                    )
            nc.scalar.copy(cqp_sb[:, :], ps_bigt[:, :])
            if n_cores > 1:
                nc.sync.dma_start(cqp_all_d[:, :], cqp_sb[:, :])
                nc.gpsimd.collective_compute(
                    "ReduceScatter",
                    mybir.AluOpType.add,
                    replica_groups=[list(range(n_cores))],
                    ins=[cqp_all_d[:, :]],
                    outs=[cqp_rs_d[:, :]],
                )
                nc.sync.dma_start(cqp_my[:, :], cqp_rs_d[:, :])
            else:
                nc.vector.tensor_copy(cqp_my[:, :], cqp_sb[0:b_c, :])
            # qw = cq_proj * W_attn
            nc.vector.tensor_mul(qw_sb[:, :], cqp_my[:, :], wattn_sb[:, :])

            # qwT: [b_c, d] -> [128(d), DC*b_c] via PE transposes (bf16)
            pq = ps_ct_p.tile([128, DC * b_c], BF16, name="pct")
            for j in range(DC):
                nc.tensor.transpose(
                    pq[:, j * b_c:(j + 1) * b_c],
                    qw_sb[:, j * 128:(j + 1) * 128],
                    idb[0:b_c, 0:b_c],
                )
            nc.vector.tensor_copy(qwT_sb[:, :], pq[:, :])

            # ---- phase 2: per-batch attention ----
            for b in range(b_c):
                nats = []
                for i in range(LT):
                    nat = natpool.tile([128, d], BF16, name=f"nat")
                    # SWDGE DMA casts f32 HBM -> bf16 SBUF in flight
                    nc.gpsimd.dma_start(nat[:, :], ctx_d[b, i * 128:(i + 1) * 128, :])
                    nats.append(nat)

                ctb = ctpool.tile([128, DC, l], BF16, name="ctb")
                for i in range(LT):
                    for g in range(NG):
                        pct = ps_ct_p.tile([128, CT_G, 128], BF16, name="pct")
                        for jj in range(CT_G):
                            j = g * CT_G + jj
                            nc.tensor.transpose(
                                pct[:, jj, :],
                                nats[i][:, j * 128:(j + 1) * 128],
                                idb[:, :],
                            )
                        dst = ctb[:, g * CT_G:(g + 1) * CT_G, i * 128:(i + 1) * 128]
                        if (i * NG + g) % 2 == 0:
                            nc.vector.tensor_copy(dst, pct[:, :, :])
                        else:
                            nc.scalar.copy(dst, pct[:, :, :])

                # logits for all queries vs this batch's context; row b is real
                ps_lg = ps_lg_p.tile([b_c, l], F32, name="ps_lg")
                for j in range(DC):
                    nc.tensor.matmul(
                        ps_lg[:, :],
                        lhsT=qwT_sb[:, j * b_c:(j + 1) * b_c],
                        rhs=ctb[:, j, :],
                        start=(j == 0),
                        stop=(j == DC - 1),
                    )

                # softmax numerator: exp(logits) * mask   (no max subtraction).
                # Full-tile ops (PSUM reads must start at partition 0); only
                # row b is meaningful, other rows are discarded downstream.
                lg_sb = lgpool.tile([b_c, l], F32, name="lg_sb")
                pm_sb = lgpool.tile([b_c, l], BF16, name="pm_sb")
                nc.scalar.activation(lg_sb[:, :], ps_lg[:, :], Exp)
                nc.vector.tensor_mul(pm_sb[:, :], lg_sb[:, :], mask_sb[:, :])

                # transpose exp-weights to l-on-partitions, drop into W8 col
                # b.  The transpose covers all b_c pm rows (ops must start at
                # partition 0); rows of other batches are garbage but their
                # columns are never read.
                pw = ps_ct_p.tile([128, LT, b_c], BF16, name="pct")
                for i in range(LT):
                    nc.tensor.transpose(
                        pw[:, i, :],
                        pm_sb[0:b_c, i * 128:(i + 1) * 128],
                        idb[0:b_c, 0:b_c],
                    )
                nc.vector.tensor_copy(W8[:, b, :, b], pw[:, :, b])

                # weighted sum + denominator
                for i in range(LT):
                    first = (b == 0 and i == 0)
                    last = (b == b_c - 1 and i == LT - 1)
                    nc.tensor.matmul(
                        ps_s[:, :],
                        lhsT=W8[:, b, i, :],
                        rhs=ones_sb[:, :],
                        start=first, stop=last,
                    )
                    for n in range(NN):
                        nc.tensor.matmul(
                            ps_bigt[0:b_c, n * 512:(n + 1) * 512],
                            lhsT=W8[:, b, i, :],
                            rhs=nats[i][:, n * 512:(n + 1) * 512],
                            start=first, stop=last,
                        )

            # ---- finalize: out = acc / denom ----
            nc.vector.reciprocal(sinv_sb[:, :], ps_s[:, :])
            nc.vector.tensor_scalar_mul(out_sb[:, :], ps_bigt[0:b_c, :],
                                        sinv_sb[:, :])
            nc.sync.dma_start(out_d[:, :], out_sb[:, :])

    nc.compile()
    return nc


def host_prep(inputs, n_cores, b_c, l, d):
    """Slice/format the full inputs into per-core input maps."""
    step = int(np.asarray(inputs["step"]))
    context = np.asarray(inputs["context"], dtype=np.float32)
    question = np.asarray(inputs["question"], dtype=np.float32)
    control_state = np.asarray(inputs["control_state"], dtype=np.float32)
    q_mask = np.asarray(inputs["q_mask"])
    W_cq = np.asarray(inputs["W_cq"], dtype=np.float32)
    b_cq = np.asarray(inputs["b_cq"], dtype=np.float32)
    step_emb = np.asarray(inputs["step_emb"], dtype=np.float32)
    W_attn = np.asarray(inputs["W_attn"], dtype=np.float32)

    bf16 = ml_dtypes.bfloat16
    d2 = 2 * d
    ksh_data = d2 // n_cores
    k_sh = ksh_data + 128
    KT = k_sh // 128

    bias = (b_cq + step_emb[step]).astype(np.float32)          # [d]
    cq = np.concatenate([control_state, question], axis=1)     # [B, 2d]
    Bfull = cq.shape[0]

    ident_bf16 = np.eye(128, dtype=bf16)
    ones_col = np.ones((128, 1), dtype=bf16)
    wattn_rep = np.broadcast_to(W_attn, (b_c, d)).astype(np.float32).copy()

    in_maps = []
    for c in range(n_cores):
        rows = slice(c * b_c, (c + 1) * b_c)
        kcols = slice(c * ksh_data, (c + 1) * ksh_data)
        # per-core K-shard of cq (all batches) + ones column (core 0 only)
        cq_aug = np.zeros((Bfull, k_sh), dtype=np.float32)
        cq_aug[:, :ksh_data] = cq[:, kcols]
        if c == 0:
            cq_aug[:, ksh_data] = 1.0
        cqT = np.ascontiguousarray(
            cq_aug.T.reshape(KT, 128, Bfull).transpose(1, 0, 2)
        ).astype(bf16)                                          # [128, KT, B]
        W_aug = np.zeros((k_sh, d), dtype=np.float32)
        W_aug[:ksh_data] = W_cq[kcols]
        if c == 0:
            W_aug[ksh_data] = bias
        in_maps.append({
            "ctx": np.ascontiguousarray(context[rows]),
            "cqT": cqT,
            "w_aug": W_aug.astype(bf16),
            "wattn": wattn_rep,
            "mask": q_mask[rows].astype(np.float32),
            "ident_bf16": ident_bf16,
            "ones_col": ones_col,
        })
    return in_maps


_NC_CACHE = {}


def _get_nc(b_c, l, d, n_cores):
    key = (b_c, l, d, n_cores)
    if key not in _NC_CACHE:
        _NC_CACHE[key] = build_nc(b_c, l, d, n_cores)
    return _NC_CACHE[key]


def kernel(**inputs) -> np.ndarray:
    context = np.asarray(inputs["context"])
    Bfull, l, d = context.shape
    n_cores = N_CORES
    b_c = Bfull // n_cores

    nc = _get_nc(b_c, l, d, n_cores)
    in_maps = host_prep(inputs, n_cores, b_c, l, d)
    res = run_bass_kernel_spmd(nc, in_maps, list(range(n_cores)))
    out = np.concatenate([res.results[c]["out"] for c in range(n_cores)], axis=0)
    return out.astype(np.float32)


# revision 19
# speedup vs baseline: 1.9311x; 1.9311x over previous
"""Trainium2 Bass kernel for the ControlUnit problem.

Computation (per batch b):
    cq      = concat([control_state, question])            # [2D]
    cq_proj = cq @ W_cq + b_cq + step_emb[step]            # [D]
    qw      = cq_proj * W_attn                             # [D]
    logits  = context[b] @ qw  (+ b_attn, softmax-invariant)
    w       = softmax(where(mask, logits, -inf))           # [L]
    out[b]  = w @ context[b]                               # [D]

Sharding: data-parallel over batch across 8 NeuronCores (8 batches/core).
All params replicated (W_cq cast to bf16 to halve its DMA traffic).

Per-core kernel structure:
  Phase 1: cq_proj matmul (bias folded in via an augmented ones-row of cq /
           bias-row of W), qw = cq_proj * W_attn, PE-transpose qw -> qwT
           so d sits on partitions.
  Phase 2 (per batch): stream context tiles [128l, D] as bf16 (cast in the
           DMA), PE-transpose them into [128d, l] chunks, matmul qwT against
           the transposed chunks to get logits, exp (no max subtraction:
           logits are ~N(0,1)), mask, PE-transpose the exp-weights into
           l-on-partition columns, then accumulate the weighted sum of the
           natural-layout tiles into a shared PSUM tile (per-batch weight
           columns are zero-padded so all 8 batches share one [8, D] PSUM
           accumulator).  The softmax denominator comes from a ones-matmul
           and is divided out once at the end.
"""

import numpy as np
import ml_dtypes
from contextlib import ExitStack

import concourse.bass as bass
import concourse.tile as tile
from concourse import bacc, mybir
from concourse.bass_utils import run_bass_kernel_spmd

F32 = mybir.dt.float32
BF16 = mybir.dt.bfloat16

N_CORES = 8
B, L, D = 64, 512, 2048


def build_nc(b_c, l, d, n_cores):
    """Build + compile the per-core Bass program (SPMD: same program on all
    cores, different data)."""
    d2a = 2 * d + 128          # augmented contraction dim (bias row block)
    KT = d2a // 128            # k-tiles for the cq_proj matmul
    LT = l // 128              # l-tiles per batch
    DC = d // 128              # 128-wide d-chunks
    NN = d // 512              # 512-wide n-chunks
    CT_G = min(8, DC)          # transposed chunks per PSUM group
    NG = DC // CT_G

    nc = bacc.Bacc("TRN2", target_bir_lowering=False, debug=False,
                   num_devices=n_cores)

    ctx_d = nc.dram_tensor("ctx", [b_c, l, d], F32, kind="ExternalInput")
    cqT_d = nc.dram_tensor("cqT", [128, KT, b_c], BF16, kind="ExternalInput")
    w_d = nc.dram_tensor("w_aug", [d2a, d], BF16, kind="ExternalInput")
    wattn_d = nc.dram_tensor("wattn", [b_c, d], F32, kind="ExternalInput")
    mask_d = nc.dram_tensor("mask", [b_c, l], F32, kind="ExternalInput")
    idb_d = nc.dram_tensor("ident_bf16", [128, 128], BF16, kind="ExternalInput")
    rowsel_d = nc.dram_tensor("rowsel", [b_c, b_c, l], mybir.dt.uint8, kind="ExternalInput")
    out_d = nc.dram_tensor("out", [b_c, d], F32, kind="ExternalOutput")

    Exp = mybir.ActivationFunctionType.Exp

    with tile.TileContext(nc) as tc:
        with ExitStack() as ctx:
            const = ctx.enter_context(tc.tile_pool(name="const", bufs=1))
            wpool = ctx.enter_context(tc.tile_pool(name="wpool", bufs=3))
            natpool = ctx.enter_context(tc.tile_pool(name="natpool", bufs=3 * LT))
            ctpool = ctx.enter_context(tc.tile_pool(name="ctpool", bufs=3))
            ps_big = ctx.enter_context(tc.tile_pool(name="ps_big", bufs=1, space="PSUM"))
            ps_lg_p = ctx.enter_context(tc.tile_pool(name="ps_lg_p", bufs=2, space="PSUM"))
            lgpool = ctx.enter_context(tc.tile_pool(name="lgpool", bufs=2))
            ps_ct_p = ctx.enter_context(tc.tile_pool(name="ps_ct_p", bufs=2, space="PSUM"))

            # ---- constants / persistent tiles ----
            idb = const.tile([128, 128], BF16)
            nc.sync.dma_start(idb[:, :], idb_d[:, :])
            rowsel_sb = const.tile([b_c, b_c, l], mybir.dt.uint8)
            nc.sync.dma_start(rowsel_sb[:, :, :], rowsel_d[:, :, :])
            cqT_sb = const.tile([128, KT, b_c], BF16)
            nc.sync.dma_start(cqT_sb[:, :, :], cqT_d[:, :, :])
            wattn_sb = const.tile([b_c, d], F32)
            nc.sync.dma_start(wattn_sb[:, :], wattn_d[:, :])
            mask_sb = const.tile([b_c, l], F32)
            nc.sync.dma_start(mask_sb[:, :], mask_d[:, :])

            qw_sb = const.tile([b_c, d], BF16)
            qwT_sb = const.tile([128, DC * b_c], BF16)
            W8 = const.tile([128, b_c, LT, b_c], BF16)
            nc.gpsimd.memset(W8[:, :, :, :], 0.0)
            pmk_sb = const.tile([b_c, l], BF16)
            ssum_sb = const.tile([b_c, 1], F32)
            sinv_sb = const.tile([b_c, 1], F32)
            out_sb = const.tile([b_c, d], F32)

            # ---- PSUM tiles ----
            # ps_bigt: cq_proj accumulator in phase 1, output accumulator in
            # phase 2 (same shape; reuse keeps us within the 8 PSUM banks).
            ps_bigt = ps_big.tile([b_c, d], F32)

            # ---- phase 1: cq_proj = cq_aug @ W_aug ----
            for k in range(KT):
                wk = wpool.tile([128, d], BF16)
                nc.sync.dma_start(wk[:, :], w_d[k * 128:(k + 1) * 128, :])
                for n in range(NN):
                    nc.tensor.matmul(
                        ps_bigt[:, n * 512:(n + 1) * 512],
                        lhsT=cqT_sb[:, k, :],
                        rhs=wk[:, n * 512:(n + 1) * 512],
                        start=(k == 0),
                        stop=(k == KT - 1),
                    )
            # qw = cq_proj * W_attn  (f32, straight out of PSUM)
            nc.vector.tensor_mul(qw_sb[:, :], ps_bigt[:, :], wattn_sb[:, :])

            # qwT: [b_c, d] -> [128(d), DC*b_c] via PE transposes (bf16)
            pq = ps_ct_p.tile([128, DC * b_c], BF16, name="pct")
            for j in range(DC):
                nc.tensor.transpose(
                    pq[:, j * b_c:(j + 1) * b_c],
                    qw_sb[:, j * 128:(j + 1) * 128],
                    idb[0:b_c, 0:b_c],
                )
            nc.vector.tensor_copy(qwT_sb[:, :], pq[:, :])

            # ---- phase 2: per-batch attention ----
            for b in range(b_c):
                nats = []
                for i in range(LT):
                    nat = natpool.tile([128, d], BF16, name=f"nat")
                    # SWDGE DMA casts f32 HBM -> bf16 SBUF in flight
                    nc.gpsimd.dma_start(nat[:, :], ctx_d[b, i * 128:(i + 1) * 128, :])
                    nats.append(nat)

                ctb = ctpool.tile([128, DC, l], BF16, name="ctb")
                for i in range(LT):
                    for g in range(NG):
                        pct = ps_ct_p.tile([128, CT_G, 128], BF16, name="pct")
                        for jj in range(CT_G):
                            j = g * CT_G + jj
                            nc.tensor.transpose(
                                pct[:, jj, :],
                                nats[i][:, j * 128:(j + 1) * 128],
                                idb[:, :],
                            )
                        dst = ctb[:, g * CT_G:(g + 1) * CT_G, i * 128:(i + 1) * 128]
                        if (i * NG + g) % 2 == 0:
                            nc.vector.tensor_copy(dst, pct[:, :, :])
                        else:
                            nc.scalar.copy(dst, pct[:, :, :])

                # logits for all queries vs this batch's context; row b is real
                ps_lg = ps_lg_p.tile([b_c, l], F32, name="ps_lg")
                for j in range(DC):
                    nc.tensor.matmul(
                        ps_lg[:, :],
                        lhsT=qwT_sb[:, j * b_c:(j + 1) * b_c],
                        rhs=ctb[:, j, :],
                        start=(j == 0),
                        stop=(j == DC - 1),
                    )

                # softmax numerator: exp(logits) * mask   (no max subtraction).
                # Full-tile ops (PSUM reads must start at partition 0); only
                # row b is meaningful, other rows are discarded downstream.
                lg_sb = lgpool.tile([b_c, l], F32, name="lg_sb")
                pm_sb = lgpool.tile([b_c, l], BF16, name="pm_sb")
                nc.scalar.activation(lg_sb[:, :], ps_lg[:, :], Exp)
                nc.vector.tensor_mul(pm_sb[:, :], lg_sb[:, :], mask_sb[:, :])
                nc.vector.copy_predicated(pmk_sb[:, :], rowsel_sb[:, b, :],
                                          pm_sb[:, :])

                # transpose exp-weights to l-on-partitions, drop into W8 col
                # b.  The transpose covers all b_c pm rows (ops must start at
                # partition 0); rows of other batches are garbage but their
                # columns are never read.
                pw = ps_ct_p.tile([128, LT, b_c], BF16, name="pct")
                for i in range(LT):
                    nc.tensor.transpose(
                        pw[:, i, :],
                        pm_sb[0:b_c, i * 128:(i + 1) * 128],
                        idb[0:b_c, 0:b_c],
                    )
                nc.vector.tensor_copy(W8[:, b, :, b], pw[:, :, b])

                # weighted sum
                for i in range(LT):
                    first = (b == 0 and i == 0)
                    last = (b == b_c - 1 and i == LT - 1)
                    for n in range(NN):
                        nc.tensor.matmul(
                            ps_bigt[:, n * 512:(n + 1) * 512],
                            lhsT=W8[:, b, i, :],
                            rhs=nats[i][:, n * 512:(n + 1) * 512],
                            start=first, stop=last,
                        )

            # ---- finalize: out = acc / denom ----
            nc.vector.reduce_sum(ssum_sb[:, :], pmk_sb[:, :],
                                 axis=mybir.AxisListType.X)
            nc.vector.reciprocal(sinv_sb[:, :], ssum_sb[:, :])
            nc.vector.tensor_scalar_mul(out_sb[:, :], ps_bigt[:, :], sinv_sb[:, :])
            nc.sync.dma_start(out_d[:, :], out_sb[:, :])

    nc.compile()
    return nc


def host_prep(inputs, n_cores, b_c, l, d):
    """Slice/format the full inputs into per-core input maps."""
    step = int(np.asarray(inputs["step"]))
    context = np.asarray(inputs["context"], dtype=np.float32)
    question = np.asarray(inputs["question"], dtype=np.float32)
    control_state = np.asarray(inputs["control_state"], dtype=np.float32)
    q_mask = np.asarray(inputs["q_mask"])
    W_cq = np.asarray(inputs["W_cq"], dtype=np.float32)
    b_cq = np.asarray(inputs["b_cq"], dtype=np.float32)
    step_emb = np.asarray(inputs["step_emb"], dtype=np.float32)
    W_attn = np.asarray(inputs["W_attn"], dtype=np.float32)

    bf16 = ml_dtypes.bfloat16
    d2 = 2 * d
    d2a = d2 + 128
    KT = d2a // 128

    bias = (b_cq + step_emb[step]).astype(np.float32)          # [d]
    cq = np.concatenate([control_state, question], axis=1)     # [B, 2d]
    Bfull = cq.shape[0]
    cq_aug = np.zeros((Bfull, d2a), dtype=np.float32)
    cq_aug[:, :d2] = cq
    cq_aug[:, d2] = 1.0
    W_aug = np.zeros((d2a, d), dtype=np.float32)
    W_aug[:d2] = W_cq
    W_aug[d2] = bias
    W_aug_bf16 = W_aug.astype(bf16)

    ident_bf16 = np.eye(128, dtype=bf16)
    rowsel = np.zeros((b_c, b_c, l), dtype=np.uint8)
    for bb in range(b_c):
        rowsel[bb, bb, :] = 1

    in_maps = []
    for c in range(n_cores):
        rows = slice(c * b_c, (c + 1) * b_c)
        cqT = np.ascontiguousarray(
            cq_aug[rows].T.reshape(KT, 128, b_c).transpose(1, 0, 2)
        ).astype(bf16)                                          # [128, KT, b_c]
        wattn_rep = np.broadcast_to(W_attn, (b_c, d)).astype(np.float32).copy()
        in_maps.append({
            "ctx": np.ascontiguousarray(context[rows]),
            "cqT": cqT,
            "w_aug": W_aug_bf16,
            "wattn": wattn_rep,
            "mask": q_mask[rows].astype(np.float32),
            "ident_bf16": ident_bf16,
            "rowsel": rowsel,
        })
    return in_maps


_NC_CACHE = {}


def _get_nc(b_c, l, d, n_cores):
    key = (b_c, l, d, n_cores)
    if key not in _NC_CACHE:
        _NC_CACHE[key] = build_nc(b_c, l, d, n_cores)
    return _NC_CACHE[key]


def kernel(**inputs) -> np.ndarray:
    context = np.asarray(inputs["context"])
    Bfull, l, d = context.shape
    n_cores = N_CORES
    b_c = Bfull // n_cores

    nc = _get_nc(b_c, l, d, n_cores)
    in_maps = host_prep(inputs, n_cores, b_c, l, d)
    res = run_bass_kernel_spmd(nc, in_maps, list(range(n_cores)))
    out = np.concatenate([res.results[c]["out"] for c in range(n_cores)], axis=0)
    return out.astype(np.float32)
